# revision 1
# baseline (speedup 1.0000x reference)
"""LSTM encoder kernel for Trainium2 (Bass/Tile), data-parallel over batch on 8 cores.

Math (per core, batch shard B=256):
  z_t = Wcat @ [x_t ; hh_{t-1}] + b      (gates pre-activation, [128, B])
  Wcat = [Wx ; 2*Whh] with g-gate columns additionally scaled by 2 so a single
  sigmoid over all 128 gate rows yields S_g = sigmoid(2 z_g), i.e.
  tanh(z_g) = 2 S_g - 1.  Reparametrize cc = c/2, hh = h/2:
    t1 = S_g - 1/2
    u  = t1 * S_i          = (i*g)/2
    v  = S_f * cc          = (f*c)/2
    cc = v + u             = c_new/2
    S_c = sigmoid(4*cc)    = sigmoid(2*c_new)
    hh = (S_c - 1/2) * S_o = o*tanh(c_new)/2 = h/2
  Host multiplies the stored hh history by 2 to recover h.

Layouts: gates on partitions (128), batch on free dim. Per chunk of TC steps one
SBUF tile [42, TC*B] holds rhs slots [x_t ; hh_{t-1}]; the hh write of step t
lands in slot t+1 (next chunk's slot 0 at boundaries). Output DMA reads rows
10:42. Partition-start rule (both-SBUF operands must share start): S_g is
relocated to start 0 (t1, GPSIMD), cc lives at start 32 (pairs with f), sigma_c
output is placed at start 96 (pairs with o). DVE runs u/cc/hh, GPSIMD runs t1/v.
"""

import numpy as np
from contextlib import ExitStack

import concourse.bass as bass
import concourse.tile as tile
from concourse import bacc, mybir
from concourse.bass_utils import run_bass_kernel_spmd

T_FULL = 512
B_FULL = 2048
IN = 10
H = 32
G = 4 * H          # 128 gate rows
K = IN + H         # 42 contraction rows of the combined matmul
NCORES = 8
B = B_FULL // NCORES  # 256 batch per core

NB = 2          # batch sub-blocks per core (latency pipelining)
FD = B // NB    # free-dim per block
TC = 16         # timesteps per SBUF chunk

DT = mybir.dt.float32
SIG = mybir.ActivationFunctionType.Sigmoid
MULT = mybir.AluOpType.mult
ADD = mybir.AluOpType.add
SUB = mybir.AluOpType.subtract

_CACHE = {}


def _build(t_total=T_FULL, tc=TC, nb=NB):
    fd = B // nb
    nchunk = t_total // tc
    nc = bacc.Bacc(trn_type="TRN2", debug=False, target_bir_lowering=False)

    xT = nc.dram_tensor("xT", [t_total, IN, B], DT, kind="ExternalInput").ap()
    wcat = nc.dram_tensor("wcat", [K, G], DT, kind="ExternalInput").ap()
    bg = nc.dram_tensor("bg", [G, 1], DT, kind="ExternalInput").ap()
    hout = nc.dram_tensor("hout", [t_total, H, B], DT, kind="ExternalOutput").ap()

    with tile.TileContext(nc) as tc_, ExitStack() as ctx:
        const = ctx.enter_context(tc_.tile_pool(name="const", bufs=1))
        xpool = ctx.enter_context(tc_.tile_pool(name="xpool", bufs=3))
        spool = ctx.enter_context(tc_.tile_pool(name="spool", bufs=4))
        cpool = ctx.enter_context(tc_.tile_pool(name="cpool", bufs=3))
        tpool = ctx.enter_context(tc_.tile_pool(name="tpool", bufs=6))
        pspool = ctx.enter_context(tc_.tile_pool(name="pspool", bufs=4, space="PSUM"))

        w_t = const.tile([K, G], DT)
        nc.sync.dma_start(w_t[:], wcat)
        bg_t = const.tile([G, 1], DT)
        nc.sync.dma_start(bg_t[:], bg)

        # rhs chunk tiles: [K, tc*B]; rows 0:H = hh slots, rows H:K = x slots
        # (hh first so its partition start is 0; x lands at start 32)
        chunk_tiles = {}

        def get_chunk(ch):
            if ch not in chunk_tiles:
                t = xpool.tile([K, tc * B], DT, name="rhs", tag="rhs")
                if ch < nchunk:
                    nc.sync.dma_start(
                        t[H:K].rearrange("p (t b) -> p t b", t=tc),
                        xT[ch * tc:(ch + 1) * tc].rearrange("t p b -> p t b"),
                    )
                chunk_tiles[ch] = t
            return chunk_tiles[ch]

        cur = get_chunk(0)
        # hh_{-1} = 0
        nc.vector.memset(cur[0:H, 0:B], 0.0)

        c_prev = []
        for blk in range(nb):
            c0 = cpool.tile([2 * H, fd], DT, name=f"cc{blk}", tag=f"cc{blk}")
            nc.vector.memset(c0[H:2 * H], 0.0)
            c_prev.append(c0)

        # Rotated software pipeline: block 1 runs half a step behind block 0,
        # so each block's PE/ACT roundtrip hides inside the other's DVE work.
        # Phase A(b, s): mm -> sigma_all -> v(GPSIMD) -> t1 -> u
        # Phase B(b, s): cc -> sigma_c -> hh
        # Tick s: A(b1, s), B(b0, s), A(b0, s+1), B(b1, s)
        state = {}

        def phase_a(blk, s_global):
            ch_, s_ = divmod(s_global, tc)
            col = s_ * B + blk * fd
            rhs = get_chunk(ch_)
            p = pspool.tile([G, fd], mybir.dt.float32, name="gates",
                            tag=f"gates{blk}")
            nc.tensor.matmul(p[:], w_t[:], rhs[:, col:col + fd],
                             start=True, stop=True)
            s_t = spool.tile([G, fd], DT, name="sgm", tag=f"sgm{blk}")
            nc.scalar.activation(s_t[:], p[:], SIG, bias=bg_t[:])
            # v = f * cc at start 32 (GPSIMD, off the DVE chain)
            v = tpool.tile([2 * H, fd], DT, name="v", tag=f"v{blk}")
            nc.gpsimd.tensor_tensor(
                v[H:2 * H], s_t[H:2 * H], c_prev[blk][H:2 * H], MULT)
            # t1 = S_g - 0.5 relocated to start 0 (DVE ts)
            t1 = tpool.tile([H, fd], DT, name="t1", tag=f"t1{blk}")
            nc.vector.tensor_scalar(t1[:], s_t[2 * H:3 * H], 0.5, None, SUB)
            # u = t1 * i (both at start 0), placed at start 32
            u = tpool.tile([2 * H, fd], DT, name="u", tag=f"u{blk}")
            nc.vector.tensor_tensor(u[H:2 * H], t1[:], s_t[0:H], MULT)
            state[blk] = (s_t, u, v, s_global)

        def phase_b(blk):
            s_t, u, v, s_global = state[blk]
            c_new = cpool.tile([2 * H, fd], DT, name=f"ccn{blk}",
                               tag=f"cc{blk}")
            nc.vector.tensor_tensor(c_new[H:2 * H], v[H:2 * H],
                                    u[H:2 * H], ADD)
            c_prev[blk] = c_new
            # sc = sigmoid(4*cc) relocated to start 96 (pairs with o)
            sc = spool.tile([G, fd], DT, name="sc", tag=f"sc{blk}")
            nc.scalar.activation(sc[3 * H:4 * H], c_new[H:2 * H],
                                 SIG, scale=4.0)
            ch_, s_ = divmod(s_global + 1, tc)
            col = s_ * B + blk * fd
            hdst = get_chunk(ch_)[0:H, col:col + fd]
            nc.vector.scalar_tensor_tensor(
                hdst, sc[3 * H:4 * H], 0.5, s_t[3 * H:4 * H], SUB, MULT)

        def emit_out(ch):
            cur_, nxt_ = get_chunk(ch), get_chunk(ch + 1)
            nc.sync.dma_start(
                hout[ch * tc:ch * tc + tc - 1].rearrange("t p b -> p t b"),
                cur_[0:H, B:].rearrange("p (t b) -> p t b", t=tc - 1),
            )
            nc.sync.dma_start(hout[ch * tc + tc - 1], nxt_[0:H, 0:B])

        phase_a(0, 0)
        for s in range(t_total):
            phase_a(1, s)
            phase_b(0)
            if s + 1 < t_total:
                phase_a(0, s + 1)
            phase_b(1)
            if s % tc == tc - 1:
                emit_out(s // tc)
    nc.compile()
    return nc


def _prep_weights(W_emb, b_emb, W_ih, W_hh, b_ih, b_hh):
    f8 = lambda a: np.asarray(a, np.float64)
    Wx = f8(W_ih) @ f8(W_emb)                                  # [G, IN]
    bgv = f8(W_ih) @ f8(b_emb) + f8(b_ih) + f8(b_hh)           # [G]
    wc = np.concatenate([2.0 * f8(W_hh).T, Wx.T], axis=0)      # [K, G] = [hh; x]
    wc[:, 2 * H:3 * H] *= 2.0
    bgv = bgv.copy()
    bgv[2 * H:3 * H] *= 2.0
    return (np.ascontiguousarray(wc.astype(np.float32)),
            np.ascontiguousarray(bgv.astype(np.float32).reshape(G, 1)))


def _run(x, W_emb, b_emb, W_ih, W_hh, b_ih, b_hh, trace=False):
    t_total = x.shape[0]
    key = (t_total, TC, NB)
    if key not in _CACHE:
        _CACHE[key] = _build(t_total, TC, NB)
    nc = _CACHE[key]

    wc, bgv = _prep_weights(W_emb, b_emb, W_ih, W_hh, b_ih, b_hh)
    x = np.asarray(x, np.float32)
    in_maps = []
    for c in range(NCORES):
        xs = np.ascontiguousarray(
            x[:, c * B:(c + 1) * B, :].transpose(0, 2, 1))  # [T, IN, B]
        in_maps.append({"xT": xs, "wcat": wc, "bg": bgv})

    res = run_bass_kernel_spmd(nc, in_maps, list(range(NCORES)), trace=trace)
    out = np.empty((t_total, B_FULL, H), np.float32)
    for c in range(NCORES):
        out[:, c * B:(c + 1) * B, :] = (
            res.results[c]["hout"].transpose(0, 2, 1) * np.float32(2.0))
    return out, res


def kernel(x, W_emb, b_emb, W_ih, W_hh, b_ih, b_hh):
    out, _ = _run(x, W_emb, b_emb, W_ih, W_hh, b_ih, b_hh, trace=False)
    return out



# revision 5
# speedup vs baseline: 1.1765x; 1.1765x over previous
"""LSTM encoder kernel for Trainium2 (Bass/Tile), data-parallel over batch on 8 cores.

Math (per core, batch shard B=256), straight LSTM (no reparametrization):
  z_t = Wcat @ [h_{t-1} ; x_t] + b     gates pre-activation, [128, B]
  gate row order [i, f, o, g] (an AP spanning >32 partitions must start at an
  aligned-to-size partition, so sigmoid covers rows 0:96 and tanh rows 96:128):
    S   = sigmoid(z_ifo + b_ifo)       (ACT, one op: Si@0, Sf@32, So@64)
    T_g = tanh(z_g + b_g)              (ACT, tanh in sigmoid_and_others table)
    u   = T_g * S_i                    (DVE, starts 0/0 -> out @32)
    v   = S_f * c_{t-1}                (Pool, starts 32/32 -> out @32)
    c_t = u + v                        (DVE, starts 32/32 -> out @32)
    tau = tanh(c_t)                    (ACT, @32 -> @64)
    h_t = S_o * tau                    (DVE, starts 64/64 -> bf16 rhs slot @0)

Precision: weights/x/h in bf16 (PE 1 cycle/row, single LDWEIGHTS), c and all
elementwise in fp32. Weights are loaded into the PE array ONCE via a standalone
ldweights; per-step matmuls are emitted with ldweights=False so the PE only
streams the rhs columns.

Layouts: gates on partitions (128), batch on free dim. Per chunk of TC steps one
SBUF tile [42, TC*B] bf16 holds rhs slots [h_{t-1} ; x_t]; the h write of step t
lands in slot t+1 (next chunk's slot 0 at boundaries). Output DMA reads rows
0:32 (bf16); host converts to fp32.
"""

import numpy as np
import ml_dtypes
from contextlib import ExitStack

import concourse.bass as bass
import concourse.tile as tile
from concourse import bacc, mybir
from concourse.bass_utils import run_bass_kernel_spmd

T_FULL = 512
B_FULL = 2048
IN = 10
H = 32
G = 4 * H          # 128 gate rows
K = IN + H         # 42 contraction rows of the combined matmul
NCORES = 8
B = B_FULL // NCORES  # 256 batch per core

NB = 2          # batch sub-blocks per core (latency pipelining)
FD = B // NB    # free-dim per block
TC = 16         # timesteps per SBUF chunk

BF16 = mybir.dt.bfloat16
F32 = mybir.dt.float32
SIG = mybir.ActivationFunctionType.Sigmoid
TANH = mybir.ActivationFunctionType.Tanh
MULT = mybir.AluOpType.mult
ADD = mybir.AluOpType.add

_CACHE = {}


def _mm_noldw(nc, out, lhsT, rhs):
    """MATMUL that reuses the PE-resident weights (no LDWEIGHTS emitted).
    Mirrors bass.BassTensor.matmul lowering with ldweights=False."""
    te = nc.tensor
    ifmap_ap = te.lower_ap(rhs.opt({0}), opt=False)
    weights_ap = te.lower_ap(lhsT.opt({0}), opt=False, for_matmul_weights=True)
    out_ap = te.lower_ap(out)
    return te.add_instruction(
        mybir.InstMatmult(
            name=te.bass.get_next_instruction_name(),
            replication_resolution=0,
            replication_shift_amnt=0,
            replication_num_rows=0,
            start_tensor_calc=True,
            stop_tensor_calc=True,
            ins=[ifmap_ap, weights_ap],
            outs=[out_ap],
            perf_mode=None,
            is_transpose=None,
            ifmap_quant_offset=None,
            weights_quant_offset=None,
            bass_skip_group_check=False,
            tile_position=None,
            tile_size=None,
            ldweights=False,
        )
    )


def _build(t_total=T_FULL, tc=TC, nb=NB):
    fd = B // nb
    nchunk = t_total // tc
    nc = bacc.Bacc(trn_type="TRN2", debug=False, target_bir_lowering=False)

    xT = nc.dram_tensor("xT", [t_total, IN, B], BF16, kind="ExternalInput").ap()
    wcat = nc.dram_tensor("wcat", [K, G], BF16, kind="ExternalInput").ap()
    bg = nc.dram_tensor("bg", [G, 1], F32, kind="ExternalInput").ap()
    hout = nc.dram_tensor("hout", [t_total, H, B], BF16, kind="ExternalOutput").ap()

    with tile.TileContext(nc) as tc_, ExitStack() as ctx:
        const = ctx.enter_context(tc_.tile_pool(name="const", bufs=1))
        xpool = ctx.enter_context(tc_.tile_pool(name="xpool", bufs=3))
        spool = ctx.enter_context(tc_.tile_pool(name="spool", bufs=4))
        taupool = ctx.enter_context(tc_.tile_pool(name="taupool", bufs=4))
        cpool = ctx.enter_context(tc_.tile_pool(name="cpool", bufs=4))
        tpool = ctx.enter_context(tc_.tile_pool(name="tpool", bufs=8))
        pspool = ctx.enter_context(tc_.tile_pool(name="pspool", bufs=4, space="PSUM"))

        w_t = const.tile([K, G], BF16)
        nc.sync.dma_start(w_t[:], wcat)
        bg_t = const.tile([G, 1], F32)
        nc.sync.dma_start(bg_t[:], bg)

        # one-time weight load; every step's matmul reuses the resident array
        nc.tensor.ldweights(w_t[:])

        # rhs chunk tiles: [K, tc*B] bf16; rows 0:H = h slots, rows H:K = x slots
        chunk_tiles = {}

        def get_chunk(ch):
            if ch not in chunk_tiles:
                t = xpool.tile([K, tc * B], BF16, name="rhs", tag="rhs")
                if ch < nchunk:
                    nc.sync.dma_start(
                        t[H:K].rearrange("p (t b) -> p t b", t=tc),
                        xT[ch * tc:(ch + 1) * tc].rearrange("t p b -> p t b"),
                    )
                chunk_tiles[ch] = t
            return chunk_tiles[ch]

        cur = get_chunk(0)
        # h_{-1} = 0
        nc.vector.memset(cur[0:H, 0:B], 0.0)

        c_prev = []
        for blk in range(nb):
            c0 = cpool.tile([2 * H, fd], F32, name=f"cc{blk}", tag=f"cc{blk}")
            nc.vector.memset(c0[H:2 * H], 0.0)
            c_prev.append(c0)

        # Rotated software pipeline: block 1 runs half a step behind block 0.
        # Phase A(b, s): mm -> sigma_ifo -> tanh_g -> v(Pool) -> u
        # Phase B(b, s): c -> tau -> h
        state = {}

        def phase_a(blk, s_global):
            ch_, s_ = divmod(s_global, tc)
            col = s_ * B + blk * fd
            rhs = get_chunk(ch_)
            p = pspool.tile([G, fd], F32, name="gates", tag=f"gates{blk}")
            _mm_noldw(nc, p[:], w_t[:], rhs[:, col:col + fd])
            # S = sigmoid(z_ifo): Si@0, Sf@32, So@64
            s_t = spool.tile([3 * H, fd], F32, name="sgm", tag=f"sgm{blk}")
            nc.scalar.activation(s_t[:], p[0:3 * H], SIG, bias=bg_t[0:3 * H])
            # T_g = tanh(z_g) relocated to start 0
            tg = tpool.tile([H, fd], F32, name="tg", tag=f"tg{blk}")
            nc.scalar.activation(tg[:], p[3 * H:4 * H], TANH,
                                 bias=bg_t[3 * H:4 * H])
            # v = f * c_prev at start 32 (Pool, off the DVE chain)
            v = tpool.tile([2 * H, fd], F32, name="v", tag=f"v{blk}")
            nc.gpsimd.tensor_tensor(
                v[H:2 * H], s_t[H:2 * H], c_prev[blk][H:2 * H], MULT)
            # u = T_g * S_i (both at start 0), placed at start 32
            u = tpool.tile([2 * H, fd], F32, name="u", tag=f"u{blk}")
            nc.vector.tensor_tensor(u[H:2 * H], tg[:], s_t[0:H], MULT)
            state[blk] = (s_t, u, v, s_global)

        def phase_b(blk):
            s_t, u, v, s_global = state[blk]
            c_new = cpool.tile([2 * H, fd], F32, name=f"ccn{blk}",
                               tag=f"cc{blk}")
            nc.vector.tensor_tensor(c_new[H:2 * H], u[H:2 * H],
                                    v[H:2 * H], ADD)
            c_prev[blk] = c_new
            # tau = tanh(c) relocated to start 64 (pairs with o)
            tau = taupool.tile([3 * H, fd], F32, name="tau", tag=f"tau{blk}")
            nc.scalar.activation(tau[2 * H:3 * H], c_new[H:2 * H], TANH)
            ch_, s_ = divmod(s_global + 1, tc)
            col = s_ * B + blk * fd
            hdst = get_chunk(ch_)[0:H, col:col + fd]
            nc.vector.tensor_tensor(hdst, s_t[2 * H:3 * H],
                                    tau[2 * H:3 * H], MULT)

        def emit_out(ch):
            cur_, nxt_ = get_chunk(ch), get_chunk(ch + 1)
            nc.sync.dma_start(
                hout[ch * tc:ch * tc + tc - 1].rearrange("t p b -> p t b"),
                cur_[0:H, B:].rearrange("p (t b) -> p t b", t=tc - 1),
            )
            nc.sync.dma_start(hout[ch * tc + tc - 1], nxt_[0:H, 0:B])

        phase_a(0, 0)
        for s in range(t_total):
            phase_a(1, s)
            phase_b(0)
            if s + 1 < t_total:
                phase_a(0, s + 1)
            phase_b(1)
            if s % tc == tc - 1:
                emit_out(s // tc)
    nc.compile()
    return nc


def _prep_weights(W_emb, b_emb, W_ih, W_hh, b_ih, b_hh):
    f8 = lambda a: np.asarray(a, np.float64)
    Wx = f8(W_ih) @ f8(W_emb)                                  # [G, IN]
    bgv = f8(W_ih) @ f8(b_emb) + f8(b_ih) + f8(b_hh)           # [G]
    perm = np.r_[0:H, H:2 * H, 3 * H:4 * H, 2 * H:3 * H]       # [i,f,o,g]
    wc = np.concatenate([f8(W_hh)[perm].T, Wx[perm].T], axis=0)  # [K, G]
    return (np.ascontiguousarray(wc.astype(ml_dtypes.bfloat16)),
            np.ascontiguousarray(bgv[perm].astype(np.float32).reshape(G, 1)))


def _run(x, W_emb, b_emb, W_ih, W_hh, b_ih, b_hh, trace=False):
    t_total = x.shape[0]
    key = (t_total, TC, NB)
    if key not in _CACHE:
        _CACHE[key] = _build(t_total, TC, NB)
    nc = _CACHE[key]

    wc, bgv = _prep_weights(W_emb, b_emb, W_ih, W_hh, b_ih, b_hh)
    x = np.asarray(x, np.float32)
    in_maps = []
    for c in range(NCORES):
        xs = np.ascontiguousarray(
            x[:, c * B:(c + 1) * B, :].transpose(0, 2, 1)).astype(
                ml_dtypes.bfloat16)  # [T, IN, B] bf16
        in_maps.append({"xT": xs, "wcat": wc, "bg": bgv})

    res = run_bass_kernel_spmd(nc, in_maps, list(range(NCORES)), trace=trace)
    out = np.empty((t_total, B_FULL, H), np.float32)
    for c in range(NCORES):
        out[:, c * B:(c + 1) * B, :] = np.asarray(
            res.results[c]["hout"], np.float32).transpose(0, 2, 1)
    return out, res


def kernel(x, W_emb, b_emb, W_ih, W_hh, b_ih, b_hh):
    out, _ = _run(x, W_emb, b_emb, W_ih, W_hh, b_ih, b_hh, trace=False)
    return out
